# revision 56
# baseline (speedup 1.0000x reference)
"""Trainium2 Bass kernel for nn_Attention_22600117911625.

Multi-head causal attention with interleaved RoPE:
  out = softmax(mask(RoPE(xWq^T) RoPE(xWk^T)^T / sqrt(128))) (xWv^T) Wo^T

Sharding over 8 NeuronCores: data-parallel over batch (2) x tensor-parallel
over 4 head-groups (4 heads each).  Per core, all matmuls in bf16:
  phase 1: Q^T/K^T (head-dim-major, de-interleave-permuted) + V projections
           from x^T; RoPE via swap-matmul + cos/sin tables.  K^T and V stay
           resident in SBUF; Q^T spills to DRAM.
  phase 2+3 fused per q-block: per head, transposed flash-style causal
           attention (S^T chunks = K^T_chunk^T Q^T, exp on ScalarE, post-exp
           0/1 tri-mask on VectorE, row-sums via bf16 DVE pair/quad
           pre-reduction + one ones-matmul per 4 k-chunks, PV in PSUM,
           1/l via ln+exp on ScalarE), then the block's full-d_model partial
           out = Wo_local A^T_local, finished by a ReduceScatter(add) over
           the 4-core group into the output.  PE-side consumers are emitted
           one pipeline stage behind their producers (rope one head behind,
           PV/ones one pair behind) so the PE FIFO never stalls on
           ScalarE/VectorE latency; the rs->out copies are scheduled last so
           nothing queues behind a collective wait.
Host side only reshapes/converts inputs and concatenates/transposes outputs.
"""
import math

import numpy as np
import ml_dtypes

import concourse.bass as bass
import concourse.mybir as mybir
from concourse import bass2jax
from concourse.tile import TileContext
from concourse.vector_clock import ScopedClock

F32 = mybir.dt.float32
BF16 = mybir.dt.bfloat16
AF = mybir.ActivationFunctionType

NP_BF16 = ml_dtypes.bfloat16

B = 2
S = 4096
DM = 2048
H = 16
DH = 128
N_CORES = 8
GROUPS = 4          # tensor-parallel head groups
HL = H // GROUPS    # heads per core (4)
EL = HL * DH        # local head width (512)
SB = 512            # q sub-block width
NSB = S // SB       # 8
# p3/ReduceScatter blocks as (first j, n j-subblocks): last blocks small so
# the tail ReduceScatter is cheap
BLOCKS = [(0, 2), (2, 2), (4, 2), (6, 1), (7, 1)]
NBLK = len(BLOCKS)
ECH = DM // 128     # 16 e-chunks
SCALE = 1.0 / math.sqrt(DH)
EXP_BIAS = -3.0
REPLICA_GROUPS = [[0, 1, 2, 3], [4, 5, 6, 7]]

_wsplit_cnt = [0]


class TC(TileContext):
    """TileContext for a walrus build that allows only ONE semaphore wait per
    instruction: extra waits are split onto nofuse NOPs on the same engine."""

    def _lower_ordered_insts(self, ordered):
        for bb_name in list(ordered.keys()):
            new = []
            for inst in ordered[bb_name]:
                si = getattr(inst, "sync_info", None)
                if si is not None and len(si.on_wait) > 1:
                    waits = list(si.on_wait)
                    eng = getattr(inst, "engine", None)
                    if eng is not None:
                        for w in waits[:-1]:
                            _wsplit_cnt[0] += 1
                            new.append(mybir.InstNoOp(
                                name=f"wsplit{_wsplit_cnt[0]}",
                                sync_info=mybir.SyncInfo(on_wait=[w], on_update=[]),
                                bass_nofuse=True,
                                engine=eng,
                            ))
                        inst.sync_info = mybir.SyncInfo(
                            on_wait=[waits[-1]], on_update=list(si.on_update))
                new.append(inst)
            ordered[bb_name] = new
        super()._lower_ordered_insts(ordered)

    def _drain_and_barrier(self, tick_clock, wait_clock):
        probe = self.nc.sync.nop(nofuse=True, hint="drain_wait_probe")
        probe.ins.sync_info = mybir.SyncInfo(on_wait=[], on_update=[])
        wait_clock.add_sem_waits(probe.ins, ScopedClock({None: tick_clock.global_clock}))
        waits = list(probe.ins.sync_info.on_wait)
        probe.ins.sync_info = mybir.SyncInfo(on_wait=waits[:1], on_update=[])
        for w in waits[1:]:
            n = self.nc.sync.nop(nofuse=True, hint="drain_wait_split")
            n.ins.sync_info = mybir.SyncInfo(on_wait=[w], on_update=[])
        self.nc.sync.drain()
        self.nc.all_engine_barrier()
        popped = self.nc._tile_sem_poison_stack.pop()
        assert popped is self._sem_poison
        self.nc.clear_and_free_semaphores(list(self.sems.allocated().values()))
        self.nc.all_engine_barrier()


def build_nc(dump=()):
    """Build the per-core SPMD kernel.  dump: subset of {"qkv", "at", "part"}
    adds debug ExternalOutputs."""
    nc = bass.Bass()

    xT = nc.declare_dram_parameter("xT", [DM, S], BF16, isOutput=False)
    wq = nc.declare_dram_parameter("wq", [128, ECH * EL], BF16, isOutput=False)
    wk = nc.declare_dram_parameter("wk", [128, ECH * EL], BF16, isOutput=False)
    wv = nc.declare_dram_parameter("wv", [128, ECH * EL], BF16, isOutput=False)
    # woT[h][d, m] = Wo[m, g*EL + h*128 + d]
    wo = nc.declare_dram_parameter("wo", [HL, 128, DM], BF16, isOutput=False)
    cosT = nc.declare_dram_parameter("cosT", [128, S], BF16, isOutput=False)
    sinT = nc.declare_dram_parameter("sinT", [128, S], BF16, isOutput=False)
    swapM = nc.declare_dram_parameter("swapM", [128, 128], BF16, isOutput=False)
    onesW = nc.declare_dram_parameter("onesW", [128, 128], BF16, isOutput=False)
    # trimask[p][k, q] = 1 if q >= k + p*128 else 0
    trimask = nc.declare_dram_parameter("trimask", [4, 128, SB], BF16, isOutput=False)
    ebias = nc.declare_dram_parameter("ebias", [128, 1], F32, isOutput=False)
    # final out: this core's 512 d_model rows (assembled from per-block RS)
    outT = nc.declare_dram_parameter("outT", [EL, S], BF16, isOutput=True)

    with TC(nc) as tc:
        with (
            tc.tile_pool(name="res", bufs=1) as resp,
            tc.tile_pool(name="dram", bufs=1, space="DRAM") as dram,
            tc.tile_pool(name="psum", bufs=2, space="PSUM") as psp,
        ):
            # --- SBUF residents (live for the whole kernel) ---
            k_res = [resp.tile([128, S], BF16, name=f"k_res{h}") for h in range(HL)]
            v_res = [resp.tile([128, S], BF16, name=f"v_res{h}") for h in range(HL)]
            wo_sb = [resp.tile([128, DM], BF16, name=f"wo_sb{h}") for h in range(HL)]
            swap_sb = resp.tile([128, 128], BF16, name="swap_sb")
            ones_sb = resp.tile([128, 128], BF16, name="ones_sb")
            tm_sb = resp.tile([128, 4 * SB], BF16, name="tm_sb")
            eb_sb = resp.tile([128, 1], F32, name="eb_sb")

            # only swap is needed at phase-1 start; the rest are issued after
            # the first x tile so they don't delay the first matmuls
            nc.sync.dma_start(out=swap_sb[:], in_=swapM[:])

            # dummy matmuls fill the initial weight/x load window and keep
            # the PE HAM clock-gate warm so the first real matmuls run at
            # full rate
            warm = psp.tile([128, 2 * SB], F32, tag="big")
            for i in range(72):
                nc.tensor.matmul(warm[:, :128], swap_sb[:], swap_sb[:],
                                 start=(i == 0), stop=(i == 71))

            def load_late_residents():
                nc.sync.dma_start(out=eb_sb[:], in_=ebias[:])
                nc.sync.dma_start(out=ones_sb[:], in_=onesW[:])
                for p in range(4):
                    nc.sync.dma_start(out=tm_sb[:, p * SB:(p + 1) * SB],
                                      in_=trimask[p])
                for h in range(HL):
                    nc.sync.dma_start(out=wo_sb[h][:], in_=wo[h])

            # --- DRAM scratch ---
            qT_d = [dram.tile([128, S], BF16, name=f"qT_d{h}") for h in range(HL)]
            part_d = [dram.tile([DM, nj * SB], BF16, name=f"part_d{b}")
                      for b, (j0, nj) in enumerate(BLOCKS)]
            rs_d = [dram.tile([EL, nj * SB], BF16, name=f"rs_d{b}")
                    for b, (j0, nj) in enumerate(BLOCKS)]

            # ---------------- phase 1 ----------------
            with (
                tc.tile_pool(name="p1x", bufs=2) as xpool,
                tc.tile_pool(name="p1w", bufs=1) as wpool,
                tc.tile_pool(name="p1st", bufs=3) as stage,
            ):
                wq_sb = wpool.tile([128, ECH * EL], BF16, tag="wq")
                nc.sync.dma_start(out=wq_sb[:], in_=wq[:])
                cos_sb = wpool.tile([128, S], BF16, tag="cos")
                nc.sync.dma_start(out=cos_sb[:], in_=cosT[:])
                sin_sb = wpool.tile([128, S], BF16, tag="sin")
                nc.sync.dma_start(out=sin_sb[:], in_=sinT[:])
                wk_sb = wpool.tile([128, ECH * EL], BF16, tag="wk")
                wv_sb = wpool.tile([128, ECH * EL], BF16, tag="wv")

                xT_r = xT[:].rearrange("(ec p) s -> p ec s", p=128)
                for sb in range(NSB):
                    ssl = slice(sb * SB, (sb + 1) * SB)
                    xt = xpool.tile([128, ECH * SB], BF16, tag="xt")
                    nc.sync.dma_start(
                        out=xt[:].rearrange("p (ec s) -> p ec s", ec=ECH),
                        in_=xT_r[:, :, ssl])
                    if sb == 0:
                        nc.sync.dma_start(out=wk_sb[:], in_=wk[:])
                        nc.sync.dma_start(out=wv_sb[:], in_=wv[:])
                        load_late_residents()

                    # Q^T / K^T head-tiles + RoPE.  The swap-matmul (which
                    # waits on the ScalarE psum->raw copy) is emitted one
                    # head group behind the projections so the PE FIFO never
                    # stalls on the copy latency.
                    def emit_rope(raw, h, is_k):
                        # psw borrows the (phase-2-only) psl tag so the
                        # p1 projection ring and rope ring stay independent
                        psw = psp.tile([128, SB], F32, tag="psl")
                        nc.tensor.matmul(psw[:], swap_sb[:], raw[:],
                                         start=True, stop=True)
                        t1 = stage.tile([128, SB], BF16, tag="t1")
                        nc.vector.tensor_mul(t1[:], raw[:], cos_sb[:, ssl])
                        t2 = stage.tile([128, SB], BF16, tag="t2")
                        nc.vector.tensor_mul(t2[:], psw[:], sin_sb[:, ssl])
                        if is_k:
                            nc.vector.tensor_add(k_res[h][:, ssl], t1[:], t2[:])
                        else:
                            qs = stage.tile([128, SB], BF16, tag="qs")
                            nc.vector.tensor_add(qs[:], t1[:], t2[:])
                            nc.sync.dma_start(out=qT_d[h][:, ssl], in_=qs[:])

                    pending = None
                    for wsb, is_k in ((wq_sb, False), (wk_sb, True)):
                        for h in range(HL):
                            ps = psp.tile([128, SB], F32, tag="psa")
                            for ec in range(ECH):
                                nc.tensor.matmul(
                                    ps[:],
                                    wsb[:, ec * EL + h * 128: ec * EL + (h + 1) * 128],
                                    xt[:, ec * SB:(ec + 1) * SB],
                                    start=(ec == 0), stop=(ec == ECH - 1))
                            raw = stage.tile([128, SB], BF16, tag="raw")
                            nc.scalar.copy(raw[:], ps[:])
                            if pending is not None:
                                emit_rope(*pending)
                            pending = (raw, h, is_k)

                    # V natural layout: lhsT = x^T chunk slice, rhs = wv
                    for st in range(4):
                        psv = psp.tile([128, 2 * SB], F32, tag="big")
                        for ec in range(ECH):
                            nc.tensor.matmul(
                                psv[:, :EL],
                                xt[:, ec * SB + st * 128: ec * SB + (st + 1) * 128],
                                wv_sb[:, ec * EL:(ec + 1) * EL],
                                start=(ec == 0), stop=(ec == ECH - 1))
                        if pending is not None:
                            emit_rope(*pending)
                            pending = None
                        stg = sb * 4 + st
                        for h in range(HL):
                            nc.scalar.copy(
                                v_res[h][:, stg * 128:(stg + 1) * 128],
                                psv[:, h * 128:(h + 1) * 128])

            # ---------------- phase 2 + 3, fused per 1024-col block ----------
            with (
                tc.tile_pool(name="p2q", bufs=4) as qpool,
                tc.tile_pool(name="p2pt", bufs=4) as ptpool,
                tc.tile_pool(name="p2st", bufs=2) as st2,
                tc.tile_pool(name="p3st", bufs=8) as st3,
                tc.tile_pool(name="p2at", bufs=6) as atpool,
            ):
                if "at" in dump:
                    adump = nc.declare_dram_parameter(
                        "atdump", [HL, 128, S], BF16, isOutput=True)
                if "linv" in dump:
                    _ld = nc.declare_dram_parameter(
                        "ldump", [HL, NSB, 128, SB], F32, isOutput=True)
                    _ldump = [[_ld[h][j] for j in range(NSB)] for h in range(HL)]
                for bi, (j0, nj) in enumerate(BLOCKS):
                    at_blk = []
                    for h in range(HL):
                        at = atpool.tile([128, nj * SB], BF16, tag="at")
                        at_blk.append(at)
                        for jj in range(nj):
                            j = j0 + jj
                            q0 = j * SB
                            qt = qpool.tile([128, SB], BF16, tag="qt")
                            nc.sync.dma_start(out=qt[:], in_=qT_d[h][:, q0:q0 + SB])
                            nk = 4 * (j + 1)
                            npair = nk // 2
                            psa = psp.tile([128, SB], F32, tag="psa")
                            psl = psp.tile([128, SB], F32, tag="psl")
                            prev_psum = None
                            quads = []   # pending quad-sum tiles for psl
                            pv_prev = None  # (pt, pr) deferred PV/ones

                            def emit_pe_consumers(pt_p, pr_p):
                                # ones-matmul for any completed quads, then
                                # the PV matmuls for pair pr_p — emitted one
                                # pair behind so the PE never waits on exp
                                while quads:
                                    qs_t, qi, qlast = quads.pop(0)
                                    nc.tensor.matmul(
                                        psl[:], ones_sb[:], qs_t[:],
                                        start=(qi == 0), stop=qlast)
                                for half in range(2):
                                    kc = 2 * pr_p + half
                                    off = (max(0, (kc - 4 * j) * 128)
                                           if kc >= 4 * j else 0)
                                    nc.tensor.matmul(
                                        psa[:, off:],
                                        v_res[h][:, kc * 128:(kc + 1) * 128],
                                        pt_p[:, half * SB + off:(half + 1) * SB],
                                        start=(kc == 0), stop=(kc == nk - 1))

                            for pr in range(npair):
                                pss = psp.tile([128, 2 * SB], F32, tag="big")
                                pt = ptpool.tile([128, 2 * SB], BF16, tag="pt")
                                for half in range(2):
                                    kc = 2 * pr + half
                                    nc.tensor.matmul(
                                        pss[:, half * SB:(half + 1) * SB],
                                        k_res[h][:, kc * 128:(kc + 1) * 128],
                                        qt[:],
                                        start=True, stop=True)
                                if pv_prev is not None:
                                    emit_pe_consumers(*pv_prev)
                                # bias -3: cancels in psa/psl; keeps bf16 sums
                                # small (legacy of the fp8 variant, harmless)
                                nc.scalar.activation(pt[:], pss[:], AF.Exp,
                                                     scale=SCALE,
                                                     bias=eb_sb[:, 0:1])
                                for half in range(2):
                                    kc = 2 * pr + half
                                    if kc >= 4 * j:
                                        p = kc - 4 * j
                                        hsl = slice(half * SB, (half + 1) * SB)
                                        nc.vector.tensor_mul(
                                            pt[:, hsl], pt[:, hsl],
                                            tm_sb[:, p * SB:(p + 1) * SB])
                                # bf16 pre-reduction for the row sums: halves
                                # of the pair summed on DVE, two pair-sums
                                # summed into a quad, one ones-matmul per quad
                                psum_t = ptpool.tile([128, SB], BF16, tag="psum")
                                nc.vector.tensor_add(psum_t[:], pt[:, :SB],
                                                     pt[:, SB:])
                                if pr % 2 == 0:
                                    prev_psum = psum_t
                                else:
                                    qsum = ptpool.tile([128, SB], BF16, tag="qsum")
                                    nc.vector.tensor_add(qsum[:], prev_psum[:],
                                                         psum_t[:])
                                    quads.append((qsum, pr // 2,
                                                  pr == npair - 1))
                                pv_prev = (pt, pr)
                            emit_pe_consumers(*pv_prev)
                            # 1/l via ln + exp(-x) on ScalarE (cheap, accurate)
                            lnl = st2.tile([128, SB], F32, tag="lnl")
                            nc.scalar.activation(lnl[:], psl[:], AF.Ln)
                            linv = st2.tile([128, SB], F32, tag="linv")
                            nc.scalar.activation(linv[:], lnl[:], AF.Exp,
                                                 scale=-1.0)
                            if "linv" in dump:
                                nc.sync.dma_start(out=_ldump[h][j], in_=linv[:])
                            nc.vector.tensor_mul(
                                at[:, jj * SB:(jj + 1) * SB], psa[:], linv[:])
                        if "at" in dump:
                            nc.sync.dma_start(
                                out=adump[h][:, j0 * SB:(j0 + nj) * SB],
                                in_=at[:])

                    # phase 3: partial out over local heads for this block
                    for dml in range(ECH):
                        for h2 in range(0, nj, 2):
                            w = min(2, nj - h2) * SB
                            po = psp.tile([128, 2 * SB], F32, tag="big")
                            for hf in range((w + SB - 1) // SB):
                                csl = slice(h2 * SB + hf * SB,
                                            h2 * SB + (hf + 1) * SB)
                                psl_ = slice(hf * SB, (hf + 1) * SB)
                                for h in range(HL):
                                    nc.tensor.matmul(
                                        po[:, psl_],
                                        wo_sb[h][:, dml * 128:(dml + 1) * 128],
                                        at_blk[h][:, csl],
                                        start=(h == 0), stop=(h == HL - 1))
                            ob = st3.tile([128, w], BF16, tag="ob")
                            nc.scalar.copy(ob[:], po[:, :w])
                            nc.scalar.dma_start(
                                out=part_d[bi][dml * 128:(dml + 1) * 128,
                                               h2 * SB:h2 * SB + w],
                                in_=ob[:])
                    nc.gpsimd.collective_compute(
                        "ReduceScatter", mybir.AluOpType.add,
                        replica_groups=REPLICA_GROUPS,
                        ins=[part_d[bi][:]],
                        outs=[rs_d[bi][:]])

                # rs_d -> outT copies scheduled LAST (tile_wait_until): they
                # wait on the RS semaphores, and anything behind them in the
                # sync-engine FIFO would stall until the collective lands
                with tc.tile_wait_until(50.0):
                    for bi, (j0, nj) in enumerate(BLOCKS):
                        nc.sync.dma_start(
                            out=outT[:, j0 * SB:(j0 + nj) * SB], in_=rs_d[bi][:])

                if "qkv" in dump:
                    qdump = nc.declare_dram_parameter(
                        "qdump", [HL, 128, S], BF16, isOutput=True)
                    kdump = nc.declare_dram_parameter(
                        "kdump", [HL, 128, S], BF16, isOutput=True)
                    vdump = nc.declare_dram_parameter(
                        "vdump", [HL, 128, S], BF16, isOutput=True)
                    for h in range(HL):
                        nc.sync.dma_start(out=qdump[h], in_=qT_d[h][:])
                        nc.sync.dma_start(out=kdump[h], in_=k_res[h][:])
                        nc.sync.dma_start(out=vdump[h], in_=v_res[h][:])
                if "part" in dump:
                    pdump = nc.declare_dram_parameter(
                        "pdump", [DM, S], BF16, isOutput=True)
                    for bi, (j0, nj) in enumerate(BLOCKS):
                        nc.sync.dma_start(
                            out=pdump[:, j0 * SB:(j0 + nj) * SB],
                            in_=part_d[bi][:])
    return nc


def _host_prep(x, Wq, Wk, Wv, Wo):
    perm = np.concatenate([np.arange(0, DH, 2), np.arange(1, DH, 2)])  # evens|odds
    rowperm = np.concatenate([h * DH + perm for h in range(HL)])

    def tile_w(Wg):  # (EL, DM) -> (128, ECH*EL): [p, ec*EL+m] = Wg[m, ec*128+p]
        return np.ascontiguousarray(
            Wg.reshape(EL, ECH, 128).transpose(2, 1, 0).reshape(128, ECH * EL)
        ).astype(NP_BF16)

    inv_freq = (1.0 / (10000.0 ** (np.arange(0, DH, 2) / DH))).astype(np.float64)
    pos = np.arange(S, dtype=np.float64)
    freqs = np.outer(inv_freq, pos)  # (64, S)
    cosT = np.concatenate([np.cos(freqs), np.cos(freqs)], 0).astype(NP_BF16)
    sinT = np.concatenate([-np.sin(freqs), np.sin(freqs)], 0).astype(NP_BF16)

    swap = np.zeros((128, 128), np.float32)
    for m in range(128):
        swap[(m + 64) % 128, m] = 1.0
    swap = swap.astype(NP_BF16)
    onesW = np.ones((128, 128), NP_BF16)
    ki = np.arange(128)[:, None]
    qi = np.arange(SB)[None, :]
    trimask = np.zeros((4, 128, SB), NP_BF16)
    for p in range(4):
        trimask[p] = (qi >= ki + p * 128).astype(NP_BF16)

    in_maps = []
    for c in range(N_CORES):
        b, g = divmod(c, GROUPS)
        sl = slice(g * EL, (g + 1) * EL)
        # woT[h][d, m] = Wo[m, g*EL + h*128 + d]
        wo_t = np.ascontiguousarray(
            Wo[:, sl].T.reshape(HL, 128, DM)).astype(NP_BF16)
        in_maps.append({
            "xT": np.ascontiguousarray(x[b].T).astype(NP_BF16),
            "wq": tile_w(Wq[sl][rowperm]),
            "wk": tile_w(Wk[sl][rowperm]),
            "wv": tile_w(Wv[sl]),
            "wo": wo_t,
            "cosT": cosT,
            "sinT": sinT,
            "swapM": swap,
            "onesW": onesW,
            "trimask": trimask,
            "ebias": np.full((128, 1), EXP_BIAS, np.float32),
        })
    return in_maps


def kernel(x, Wq, Wk, Wv, Wo):
    in_maps = _host_prep(x, Wq, Wk, Wv, Wo)
    nc = build_nc()
    res = bass2jax.run_bass_via_pjrt(nc, in_maps, n_cores=N_CORES)
    out = np.empty((B, S, DM), np.float32)
    for c in range(N_CORES):
        b, g = divmod(c, GROUPS)
        out[b, :, g * EL:(g + 1) * EL] = res[c]["outT"].astype(np.float32).T
    return out


if __name__ == "__main__":
    rng = np.random.default_rng(0)
    x = rng.standard_normal((B, S, DM)).astype(np.float32)
    Wq = (rng.standard_normal((H * DH, DM)) * 0.02).astype(np.float32)
    Wk = (rng.standard_normal((H * DH, DM)) * 0.02).astype(np.float32)
    Wv = (rng.standard_normal((H * DH, DM)) * 0.02).astype(np.float32)
    Wo = (rng.standard_normal((DM, H * DH)) * 0.02).astype(np.float32)
    out = kernel(x, Wq, Wk, Wv, Wo)
    print(out.shape, out.dtype)


# revision 59
# speedup vs baseline: 1.0620x; 1.0620x over previous
"""Trainium2 Bass kernel for nn_Attention_22600117911625.

Multi-head causal attention with interleaved RoPE:
  out = softmax(mask(RoPE(xWq^T) RoPE(xWk^T)^T / sqrt(128))) (xWv^T) Wo^T

Sharding over 8 NeuronCores: data-parallel over batch (2) x tensor-parallel
over 4 head-groups (4 heads each).  Per core, all matmuls in bf16:
  phase 1: Q^T/K^T (head-dim-major, de-interleave-permuted) + V projections
           from x^T; RoPE via swap-matmul + cos/sin tables.  K^T and V stay
           resident in SBUF; Q^T spills to DRAM.
  phase 2+3 fused per q-block: per head, transposed flash-style causal
           attention (S^T chunks = K^T_chunk^T Q^T, exp on ScalarE, post-exp
           0/1 tri-mask on VectorE, row-sums via bf16 DVE pair/quad
           pre-reduction + one ones-matmul per 4 k-chunks, PV in PSUM,
           1/l via ln+exp on ScalarE), then the block's full-d_model partial
           out = Wo_local A^T_local, finished by a ReduceScatter(add) over
           the 4-core group into the output.  PE-side consumers are emitted
           one pipeline stage behind their producers (rope one head behind,
           PV/ones one pair behind) so the PE FIFO never stalls on
           ScalarE/VectorE latency; the rs->out copies are scheduled last so
           nothing queues behind a collective wait.
Host side only reshapes/converts inputs and concatenates/transposes outputs.
"""
import math

import numpy as np
import ml_dtypes

import concourse.bass as bass
import concourse.mybir as mybir
from concourse import bass2jax
from concourse.tile import TileContext
from concourse.vector_clock import ScopedClock

F32 = mybir.dt.float32
BF16 = mybir.dt.bfloat16
AF = mybir.ActivationFunctionType

NP_BF16 = ml_dtypes.bfloat16

B = 2
S = 4096
DM = 2048
H = 16
DH = 128
N_CORES = 8
GROUPS = 4          # tensor-parallel head groups
HL = H // GROUPS    # heads per core (4)
EL = HL * DH        # local head width (512)
SB = 512            # q sub-block width
NSB = S // SB       # 8
# p3/ReduceScatter blocks as (first j, n j-subblocks): last blocks small so
# the tail ReduceScatter is cheap
BLOCKS = [(0, 2), (2, 2), (4, 2), (6, 1), (7, 1)]
NBLK = len(BLOCKS)
ECH = DM // 128     # 16 e-chunks
SCALE = 1.0 / math.sqrt(DH)
EXP_BIAS = -3.0
REPLICA_GROUPS = [[0, 1, 2, 3], [4, 5, 6, 7]]

_wsplit_cnt = [0]


class TC(TileContext):
    """TileContext for a walrus build that allows only ONE semaphore wait per
    instruction: extra waits are split onto nofuse NOPs on the same engine."""

    def _lower_ordered_insts(self, ordered):
        for bb_name in list(ordered.keys()):
            new = []
            for inst in ordered[bb_name]:
                si = getattr(inst, "sync_info", None)
                if si is not None and len(si.on_wait) > 1:
                    waits = list(si.on_wait)
                    eng = getattr(inst, "engine", None)
                    if eng is not None:
                        for w in waits[:-1]:
                            _wsplit_cnt[0] += 1
                            new.append(mybir.InstNoOp(
                                name=f"wsplit{_wsplit_cnt[0]}",
                                sync_info=mybir.SyncInfo(on_wait=[w], on_update=[]),
                                bass_nofuse=True,
                                engine=eng,
                            ))
                        inst.sync_info = mybir.SyncInfo(
                            on_wait=[waits[-1]], on_update=list(si.on_update))
                new.append(inst)
            ordered[bb_name] = new
        super()._lower_ordered_insts(ordered)

    def _drain_and_barrier(self, tick_clock, wait_clock):
        probe = self.nc.sync.nop(nofuse=True, hint="drain_wait_probe")
        probe.ins.sync_info = mybir.SyncInfo(on_wait=[], on_update=[])
        wait_clock.add_sem_waits(probe.ins, ScopedClock({None: tick_clock.global_clock}))
        waits = list(probe.ins.sync_info.on_wait)
        probe.ins.sync_info = mybir.SyncInfo(on_wait=waits[:1], on_update=[])
        for w in waits[1:]:
            n = self.nc.sync.nop(nofuse=True, hint="drain_wait_split")
            n.ins.sync_info = mybir.SyncInfo(on_wait=[w], on_update=[])
        self.nc.sync.drain()
        self.nc.all_engine_barrier()
        popped = self.nc._tile_sem_poison_stack.pop()
        assert popped is self._sem_poison
        self.nc.clear_and_free_semaphores(list(self.sems.allocated().values()))
        self.nc.all_engine_barrier()


def build_nc(dump=()):
    """Build the per-core SPMD kernel.  dump: subset of {"qkv", "at", "part"}
    adds debug ExternalOutputs."""
    nc = bass.Bass()

    xT = nc.declare_dram_parameter("xT", [DM, S], BF16, isOutput=False)
    wq = nc.declare_dram_parameter("wq", [128, ECH * EL], BF16, isOutput=False)
    wk = nc.declare_dram_parameter("wk", [128, ECH * EL], BF16, isOutput=False)
    wv = nc.declare_dram_parameter("wv", [128, ECH * EL], BF16, isOutput=False)
    # woT[h][d, m] = Wo[m, g*EL + h*128 + d]
    wo = nc.declare_dram_parameter("wo", [HL, 128, DM], BF16, isOutput=False)
    cosT = nc.declare_dram_parameter("cosT", [128, S], BF16, isOutput=False)
    sinT = nc.declare_dram_parameter("sinT", [128, S], BF16, isOutput=False)
    swapM = nc.declare_dram_parameter("swapM", [128, 128], BF16, isOutput=False)
    onesW = nc.declare_dram_parameter("onesW", [128, 128], BF16, isOutput=False)
    # trimask[p][k, q] = 1 if q >= k + p*128 else 0
    trimask = nc.declare_dram_parameter("trimask", [4, 128, SB], BF16, isOutput=False)
    ebias = nc.declare_dram_parameter("ebias", [128, 1], F32, isOutput=False)
    # final out: this core's 512 d_model rows (assembled from per-block RS)
    outT = nc.declare_dram_parameter("outT", [EL, S], BF16, isOutput=True)

    with TC(nc) as tc:
        with (
            tc.tile_pool(name="res", bufs=1) as resp,
            tc.tile_pool(name="dram", bufs=1, space="DRAM") as dram,
            tc.tile_pool(name="psum", bufs=2, space="PSUM") as psp,
        ):
            # --- SBUF residents (live for the whole kernel) ---
            k_res = [resp.tile([128, S], BF16, name=f"k_res{h}") for h in range(HL)]
            v_res = [resp.tile([128, S], BF16, name=f"v_res{h}") for h in range(HL)]
            wo_sb = [resp.tile([128, DM], BF16, name=f"wo_sb{h}") for h in range(HL)]
            swap_sb = resp.tile([128, 128], BF16, name="swap_sb")
            ones_sb = resp.tile([128, 128], BF16, name="ones_sb")
            tm_sb = resp.tile([128, 4 * SB], BF16, name="tm_sb")
            eb_sb = resp.tile([128, 1], F32, name="eb_sb")

            # only swap is needed at phase-1 start; the rest are issued after
            # the first x tile so they don't delay the first matmuls
            nc.sync.dma_start(out=swap_sb[:], in_=swapM[:])

            def load_late_residents():
                nc.sync.dma_start(out=eb_sb[:], in_=ebias[:])
                nc.sync.dma_start(out=ones_sb[:], in_=onesW[:])
                for p in range(4):
                    nc.sync.dma_start(out=tm_sb[:, p * SB:(p + 1) * SB],
                                      in_=trimask[p])
                for h in range(HL):
                    nc.sync.dma_start(out=wo_sb[h][:], in_=wo[h])

            # --- DRAM scratch ---
            qT_d = [dram.tile([128, S], BF16, name=f"qT_d{h}") for h in range(HL)]
            part_d = [dram.tile([DM, nj * SB], BF16, name=f"part_d{b}")
                      for b, (j0, nj) in enumerate(BLOCKS)]
            rs_d = [dram.tile([EL, nj * SB], BF16, name=f"rs_d{b}")
                    for b, (j0, nj) in enumerate(BLOCKS)]

            # ---------------- phase 1 ----------------
            with (
                tc.tile_pool(name="p1x", bufs=2) as xpool,
                tc.tile_pool(name="p1w", bufs=1) as wpool,
                tc.tile_pool(name="p1st", bufs=3) as stage,
            ):
                wq_sb = wpool.tile([128, ECH * EL], BF16, tag="wq")
                nc.sync.dma_start(out=wq_sb[:], in_=wq[:])
                cos_sb = wpool.tile([128, S], BF16, tag="cos")
                nc.sync.dma_start(out=cos_sb[:], in_=cosT[:])
                sin_sb = wpool.tile([128, S], BF16, tag="sin")
                nc.sync.dma_start(out=sin_sb[:], in_=sinT[:])
                wk_sb = wpool.tile([128, ECH * EL], BF16, tag="wk")
                wv_sb = wpool.tile([128, ECH * EL], BF16, tag="wv")

                xT_r = xT[:].rearrange("(ec p) s -> p ec s", p=128)
                for sb in range(NSB):
                    ssl = slice(sb * SB, (sb + 1) * SB)
                    xt = xpool.tile([128, ECH * SB], BF16, tag="xt")
                    nc.sync.dma_start(
                        out=xt[:].rearrange("p (ec s) -> p ec s", ec=ECH),
                        in_=xT_r[:, :, ssl])
                    if sb == 0:
                        nc.sync.dma_start(out=wk_sb[:], in_=wk[:])
                        nc.sync.dma_start(out=wv_sb[:], in_=wv[:])
                        load_late_residents()

                    # Q^T / K^T head-tiles + RoPE.  The swap-matmul (which
                    # waits on the ScalarE psum->raw copy) is emitted one
                    # head group behind the projections so the PE FIFO never
                    # stalls on the copy latency.
                    def emit_rope(raw, h, is_k):
                        # psw borrows the (phase-2-only) psl tag so the
                        # p1 projection ring and rope ring stay independent
                        psw = psp.tile([128, SB], F32, tag="psl")
                        nc.tensor.matmul(psw[:], swap_sb[:], raw[:],
                                         start=True, stop=True)
                        t1 = stage.tile([128, SB], BF16, tag="t1")
                        nc.vector.tensor_mul(t1[:], raw[:], cos_sb[:, ssl])
                        t2 = stage.tile([128, SB], BF16, tag="t2")
                        nc.vector.tensor_mul(t2[:], psw[:], sin_sb[:, ssl])
                        if is_k:
                            nc.vector.tensor_add(k_res[h][:, ssl], t1[:], t2[:])
                        else:
                            qs = stage.tile([128, SB], BF16, tag="qs")
                            nc.vector.tensor_add(qs[:], t1[:], t2[:])
                            nc.sync.dma_start(out=qT_d[h][:, ssl], in_=qs[:])

                    pending = None
                    for wsb, is_k in ((wq_sb, False), (wk_sb, True)):
                        for h in range(HL):
                            ps = psp.tile([128, SB], F32, tag="psa")
                            for ec in range(ECH):
                                nc.tensor.matmul(
                                    ps[:],
                                    wsb[:, ec * EL + h * 128: ec * EL + (h + 1) * 128],
                                    xt[:, ec * SB:(ec + 1) * SB],
                                    start=(ec == 0), stop=(ec == ECH - 1))
                            raw = stage.tile([128, SB], BF16, tag="raw")
                            nc.scalar.copy(raw[:], ps[:])
                            if pending is not None:
                                emit_rope(*pending)
                            pending = (raw, h, is_k)

                    # V natural layout: lhsT = x^T chunk slice, rhs = wv
                    for st in range(4):
                        psv = psp.tile([128, 2 * SB], F32, tag="big")
                        for ec in range(ECH):
                            nc.tensor.matmul(
                                psv[:, :EL],
                                xt[:, ec * SB + st * 128: ec * SB + (st + 1) * 128],
                                wv_sb[:, ec * EL:(ec + 1) * EL],
                                start=(ec == 0), stop=(ec == ECH - 1))
                        if pending is not None:
                            emit_rope(*pending)
                            pending = None
                        stg = sb * 4 + st
                        for h in range(HL):
                            nc.scalar.copy(
                                v_res[h][:, stg * 128:(stg + 1) * 128],
                                psv[:, h * 128:(h + 1) * 128])

            # ---------------- phase 2 + 3, fused per 1024-col block ----------
            with (
                tc.tile_pool(name="p2q", bufs=4) as qpool,
                tc.tile_pool(name="p2pt", bufs=4) as ptpool,
                tc.tile_pool(name="p2st", bufs=2) as st2,
                tc.tile_pool(name="p3st", bufs=8) as st3,
                tc.tile_pool(name="p2at", bufs=6) as atpool,
            ):
                if "at" in dump:
                    adump = nc.declare_dram_parameter(
                        "atdump", [HL, 128, S], BF16, isOutput=True)
                if "linv" in dump:
                    _ld = nc.declare_dram_parameter(
                        "ldump", [HL, NSB, 128, SB], F32, isOutput=True)
                    _ldump = [[_ld[h][j] for j in range(NSB)] for h in range(HL)]
                for bi, (j0, nj) in enumerate(BLOCKS):
                    at_blk = []
                    for h in range(HL):
                        at = atpool.tile([128, nj * SB], BF16, tag="at")
                        at_blk.append(at)
                        for jj in range(nj):
                            j = j0 + jj
                            q0 = j * SB
                            qt = qpool.tile([128, SB], BF16, tag="qt")
                            nc.sync.dma_start(out=qt[:], in_=qT_d[h][:, q0:q0 + SB])
                            nk = 4 * (j + 1)
                            npair = nk // 2
                            psa = psp.tile([128, SB], F32, tag="psa")
                            psl = psp.tile([128, SB], F32, tag="psl")
                            prev_psum = None
                            quads = []   # pending quad-sum tiles for psl
                            pv_prev = None  # (pt, pr) deferred PV/ones

                            def emit_pe_consumers(pt_p, pr_p):
                                # ones-matmul for any completed quads, then
                                # the PV matmuls for pair pr_p — emitted one
                                # pair behind so the PE never waits on exp
                                while quads:
                                    qs_t, qi, qlast = quads.pop(0)
                                    nc.tensor.matmul(
                                        psl[:], ones_sb[:], qs_t[:],
                                        start=(qi == 0), stop=qlast)
                                for half in range(2):
                                    kc = 2 * pr_p + half
                                    off = (max(0, (kc - 4 * j) * 128)
                                           if kc >= 4 * j else 0)
                                    nc.tensor.matmul(
                                        psa[:, off:],
                                        v_res[h][:, kc * 128:(kc + 1) * 128],
                                        pt_p[:, half * SB + off:(half + 1) * SB],
                                        start=(kc == 0), stop=(kc == nk - 1))

                            for pr in range(npair):
                                pss = psp.tile([128, 2 * SB], F32, tag="big")
                                pt = ptpool.tile([128, 2 * SB], BF16, tag="pt")
                                for half in range(2):
                                    kc = 2 * pr + half
                                    nc.tensor.matmul(
                                        pss[:, half * SB:(half + 1) * SB],
                                        k_res[h][:, kc * 128:(kc + 1) * 128],
                                        qt[:],
                                        start=True, stop=True)
                                if pv_prev is not None:
                                    emit_pe_consumers(*pv_prev)
                                # bias -3: cancels in psa/psl; keeps bf16 sums
                                # small (legacy of the fp8 variant, harmless)
                                nc.scalar.activation(pt[:], pss[:], AF.Exp,
                                                     scale=SCALE,
                                                     bias=eb_sb[:, 0:1])
                                for half in range(2):
                                    kc = 2 * pr + half
                                    if kc >= 4 * j:
                                        p = kc - 4 * j
                                        hsl = slice(half * SB, (half + 1) * SB)
                                        nc.vector.tensor_mul(
                                            pt[:, hsl], pt[:, hsl],
                                            tm_sb[:, p * SB:(p + 1) * SB])
                                # bf16 pre-reduction for the row sums: halves
                                # of the pair summed on DVE, two pair-sums
                                # summed into a quad, one ones-matmul per quad
                                psum_t = ptpool.tile([128, SB], BF16, tag="psum")
                                nc.vector.tensor_add(psum_t[:], pt[:, :SB],
                                                     pt[:, SB:])
                                if pr % 2 == 0:
                                    prev_psum = psum_t
                                else:
                                    qsum = ptpool.tile([128, SB], BF16, tag="qsum")
                                    nc.vector.tensor_add(qsum[:], prev_psum[:],
                                                         psum_t[:])
                                    quads.append((qsum, pr // 2,
                                                  pr == npair - 1))
                                pv_prev = (pt, pr)
                            emit_pe_consumers(*pv_prev)
                            # 1/l via ln + exp(-x) on ScalarE (cheap, accurate)
                            lnl = st2.tile([128, SB], F32, tag="lnl")
                            nc.scalar.activation(lnl[:], psl[:], AF.Ln)
                            linv = st2.tile([128, SB], F32, tag="linv")
                            nc.scalar.activation(linv[:], lnl[:], AF.Exp,
                                                 scale=-1.0)
                            if "linv" in dump:
                                nc.sync.dma_start(out=_ldump[h][j], in_=linv[:])
                            nc.vector.tensor_mul(
                                at[:, jj * SB:(jj + 1) * SB], psa[:], linv[:])
                        if "at" in dump:
                            nc.sync.dma_start(
                                out=adump[h][:, j0 * SB:(j0 + nj) * SB],
                                in_=at[:])

                    # phase 3: partial out over local heads for this block
                    for dml in range(ECH):
                        for h2 in range(0, nj, 2):
                            w = min(2, nj - h2) * SB
                            po = psp.tile([128, 2 * SB], F32, tag="big")
                            for hf in range((w + SB - 1) // SB):
                                csl = slice(h2 * SB + hf * SB,
                                            h2 * SB + (hf + 1) * SB)
                                psl_ = slice(hf * SB, (hf + 1) * SB)
                                for h in range(HL):
                                    nc.tensor.matmul(
                                        po[:, psl_],
                                        wo_sb[h][:, dml * 128:(dml + 1) * 128],
                                        at_blk[h][:, csl],
                                        start=(h == 0), stop=(h == HL - 1))
                            ob = st3.tile([128, w], BF16, tag="ob")
                            nc.scalar.copy(ob[:], po[:, :w])
                            nc.sync.dma_start(
                                out=part_d[bi][dml * 128:(dml + 1) * 128,
                                               h2 * SB:h2 * SB + w],
                                in_=ob[:])
                    nc.gpsimd.collective_compute(
                        "ReduceScatter", mybir.AluOpType.add,
                        replica_groups=REPLICA_GROUPS,
                        ins=[part_d[bi][:]],
                        outs=[rs_d[bi][:]])

                # rs_d -> outT copies scheduled LAST (tile_wait_until): they
                # wait on the RS semaphores, and anything behind them in the
                # sync-engine FIFO would stall until the collective lands
                with tc.tile_wait_until(50.0):
                    for bi, (j0, nj) in enumerate(BLOCKS):
                        nc.sync.dma_start(
                            out=outT[:, j0 * SB:(j0 + nj) * SB], in_=rs_d[bi][:])

                if "qkv" in dump:
                    qdump = nc.declare_dram_parameter(
                        "qdump", [HL, 128, S], BF16, isOutput=True)
                    kdump = nc.declare_dram_parameter(
                        "kdump", [HL, 128, S], BF16, isOutput=True)
                    vdump = nc.declare_dram_parameter(
                        "vdump", [HL, 128, S], BF16, isOutput=True)
                    for h in range(HL):
                        nc.sync.dma_start(out=qdump[h], in_=qT_d[h][:])
                        nc.sync.dma_start(out=kdump[h], in_=k_res[h][:])
                        nc.sync.dma_start(out=vdump[h], in_=v_res[h][:])
                if "part" in dump:
                    pdump = nc.declare_dram_parameter(
                        "pdump", [DM, S], BF16, isOutput=True)
                    for bi, (j0, nj) in enumerate(BLOCKS):
                        nc.sync.dma_start(
                            out=pdump[:, j0 * SB:(j0 + nj) * SB],
                            in_=part_d[bi][:])
    return nc


def _host_prep(x, Wq, Wk, Wv, Wo):
    perm = np.concatenate([np.arange(0, DH, 2), np.arange(1, DH, 2)])  # evens|odds
    rowperm = np.concatenate([h * DH + perm for h in range(HL)])

    def tile_w(Wg):  # (EL, DM) -> (128, ECH*EL): [p, ec*EL+m] = Wg[m, ec*128+p]
        return np.ascontiguousarray(
            Wg.reshape(EL, ECH, 128).transpose(2, 1, 0).reshape(128, ECH * EL)
        ).astype(NP_BF16)

    inv_freq = (1.0 / (10000.0 ** (np.arange(0, DH, 2) / DH))).astype(np.float64)
    pos = np.arange(S, dtype=np.float64)
    freqs = np.outer(inv_freq, pos)  # (64, S)
    cosT = np.concatenate([np.cos(freqs), np.cos(freqs)], 0).astype(NP_BF16)
    sinT = np.concatenate([-np.sin(freqs), np.sin(freqs)], 0).astype(NP_BF16)

    swap = np.zeros((128, 128), np.float32)
    for m in range(128):
        swap[(m + 64) % 128, m] = 1.0
    swap = swap.astype(NP_BF16)
    onesW = np.ones((128, 128), NP_BF16)
    ki = np.arange(128)[:, None]
    qi = np.arange(SB)[None, :]
    trimask = np.zeros((4, 128, SB), NP_BF16)
    for p in range(4):
        trimask[p] = (qi >= ki + p * 128).astype(NP_BF16)

    in_maps = []
    for c in range(N_CORES):
        b, g = divmod(c, GROUPS)
        sl = slice(g * EL, (g + 1) * EL)
        # woT[h][d, m] = Wo[m, g*EL + h*128 + d]
        wo_t = np.ascontiguousarray(
            Wo[:, sl].T.reshape(HL, 128, DM)).astype(NP_BF16)
        in_maps.append({
            "xT": np.ascontiguousarray(x[b].T).astype(NP_BF16),
            "wq": tile_w(Wq[sl][rowperm]),
            "wk": tile_w(Wk[sl][rowperm]),
            "wv": tile_w(Wv[sl]),
            "wo": wo_t,
            "cosT": cosT,
            "sinT": sinT,
            "swapM": swap,
            "onesW": onesW,
            "trimask": trimask,
            "ebias": np.full((128, 1), EXP_BIAS, np.float32),
        })
    return in_maps


def kernel(x, Wq, Wk, Wv, Wo):
    in_maps = _host_prep(x, Wq, Wk, Wv, Wo)
    nc = build_nc()
    res = bass2jax.run_bass_via_pjrt(nc, in_maps, n_cores=N_CORES)
    out = np.empty((B, S, DM), np.float32)
    for c in range(N_CORES):
        b, g = divmod(c, GROUPS)
        out[b, :, g * EL:(g + 1) * EL] = res[c]["outT"].astype(np.float32).T
    return out


if __name__ == "__main__":
    rng = np.random.default_rng(0)
    x = rng.standard_normal((B, S, DM)).astype(np.float32)
    Wq = (rng.standard_normal((H * DH, DM)) * 0.02).astype(np.float32)
    Wk = (rng.standard_normal((H * DH, DM)) * 0.02).astype(np.float32)
    Wv = (rng.standard_normal((H * DH, DM)) * 0.02).astype(np.float32)
    Wo = (rng.standard_normal((DM, H * DH)) * 0.02).astype(np.float32)
    out = kernel(x, Wq, Wk, Wv, Wo)
    print(out.shape, out.dtype)
